# revision 46
# baseline (speedup 1.0000x reference)
"""Causal self-attention kernel for 8 Trainium2 NeuronCores.

Problem: B=2, T=2048, D=1024, 16 heads x 64. Tensor-parallel over heads:
core c owns heads [2c, 2c+1] (128 projection columns), computes its partial
output through wo's matching 128 rows; host sums the 8 partials (TP
all-reduce done at unshard time).

All compute in bf16 (fp32 PSUM accumulation). Device-side dataflow per core:
  - x pre-transposed on host to xT [D, B*T] bf16; weights pre-arranged on
    host to [p, a, n] k-tile layout so every DMA line is 2KB.
  - Q^T/K^T/V^T projections: stationary = weight k-tile [128,128], moving
    = xT chunk; q/k/v share one 3-bank PSUM tile [128,1536] -> single DVE
    cast per 512-chunk into qkv_T [128, 3, BT] bf16.
  - V^T is transposed 128x128-wise on the PE into V_ext tiles [s=128,193]
    whose cols 64/65 are ones: head0 AV lhsT = cols 0:65 (denom at psum
    part 64), head1 AV lhsT = cols 65:193 (denom at part 0, O at 64:128).
  - scores: both heads share one [128, 2, 512] PSUM tile; the two K=64
    matmuls sit on disjoint PE row groups -> run concurrently. Causal mask
    added as a bf16 identity@mask matmul (additive -448) on the diagonal
    block only; fully masked chunks skipped.
  - exp is split across TWO engines to break the ACT bottleneck:
    full s-tiles -> ACT (activation Exp, scale=1/8); diagonal s-tiles
    (r>=1) -> DVE via two custom ops: EXP2P_ANT evaluates a deg-4
    polynomial p(s) ~ 2^(lam*s/128) (p(0)=1 constrained, lam =
    log2(e)/8), EXP2SQ_ANT raises p^128 by 7 squarings. End-to-end
    rel err ~0.4% max, same order as the bf16 cast. Mask constant is
    -448 (not -1e30) so the polynomial domain stays bounded.
  - PE issue order per t-chunk is software-pipelined: scores(si+1) is
    issued BEFORE AV(si), so the PE never head-of-line blocks on the
    exp of the current s-tile (engine queues are strict FIFO).
  - normalize is fused into the AV-psum drain: denominator slivers ->
    K=65 selector matmul broadcast -> reciprocal_approx_fast ->
    tensor_tensor mult (psum x rb -> oT2 bf16). The selector matmul for
    t-chunk tj is deferred into tj+1's stream (after scores si=1) so the
    PE does not stall on the DVE sliver casts.
  - out projection: both heads in one K=128 contraction; the two 512-col
    psum banks drain with ONE merged [128,2,512] cast, alternating
    DVE/DVE/ACT by tile; output DMA'd as bf16 per 128-row tile.

Hard-won HW rules: never let two matmuls with disjoint stationary row
groups accumulate into the same PSUM bank (concurrent drains collide,
fatal). Keep per-engine instruction streams dense and dependency-ready in
emission order: the engine queues are FIFO, so an instruction whose
inputs aren't ready head-of-line-blocks everything behind it.
"""

import sys

if "/opt/trn_rl_repo" not in sys.path:
    sys.path.insert(0, "/opt/trn_rl_repo")

import numpy as np
import ml_dtypes

BF = ml_dtypes.bfloat16

N_HEAD = 16
D_HEAD = 64
D = 1024
B = 2
T = 2048
NCORES = 8
HPC = N_HEAD // NCORES          # heads per core = 2
DC = HPC * D_HEAD               # projection cols per core = 128
BT = B * T                      # 4096
TCH = 512                       # t-chunk (PSUM bank = 512 fp32)
XCH = 1024                      # x DMA chunk (2KB bf16 lines)
NTCH = T // TCH                 # 4 t-chunks per batch
NST = T // 128                  # 16 s-tiles per batch

MASKVAL = -448.0                # additive causal mask (bounded for DVE poly)

# deg-4 minimax fit of 2^(c*s) on s in [-560, 100], c = log2(e)/8/128,
# constrained p(0)=1; exp(s/8) = p(s)^128 via 7 squarings.
EXP_C = [0.0009765843191788203, 4.7669143663329587e-07,
         1.5284584395924936e-10, 3.046271572502866e-14]

_cache = {}


def _register_exp_ops():
    """Register the two custom DVE ops (idempotent)."""
    from concourse import dve_ops
    from concourse.dve_spec import (
        Spec, Src0, C0, C1, C2, C3, One, sq, lower, _spill_c3_to_src1,
        _has_src1,
    )
    from concourse.dve_uop import DveOpSpec

    def mk(name, body, reference):
        if name in dve_ops._SUB_OPCODE_FOR_NAME:
            return next(op for op in dve_ops.OPS if op.name == name)
        spec = Spec(body=body, reference=reference)
        shas = {}
        for ver in ("v3", "v4"):
            s = DveOpSpec(name=name, opcode=1, uops=lower(spec, ver=ver),
                          rd1_en=_has_src1(spec))
            shas[ver] = s.sha(ver)
        op = dve_ops.DveOp(name, spec, subdim=False, uops_sha=shas)
        dve_ops.OPS.append(op)
        dve_ops._SUB_OPCODE_FOR_NAME[name] = (
            dve_ops._CUSTOM_DVE_ROW_BASE + len(dve_ops.OPS) - 1
        )
        dve_ops.CUSTOM_DVE_SPECS[name] = spec
        return op

    _p1 = One + Src0 * (C0 + Src0 * (C1 + Src0 * (C2 + Src0 * C3)))
    P1 = mk(
        "EXP2P_ANT",
        _spill_c3_to_src1(_p1),
        lambda in0, in1, s0, s1, imm2: 1.0
        + in0 * (s0 + in0 * (s1 + in0 * (imm2 + in0 * np.asarray(in1).ravel()[0]))),
    )
    _p2 = Src0
    for _ in range(7):
        _p2 = sq(_p2)
    P2 = mk("EXP2SQ_ANT", _p2, lambda in0, in1, s0, s1, imm2: in0 ** 128)
    return P1, P2


def _use_dve_exp(b, tj, r):
    """Which s-tiles' exp runs on DVE (custom ops) vs ACT."""
    return r >= 2 or (r == 1 and b == 0)


def _build():
    import concourse.bass as bass
    import concourse.mybir as mybir
    import concourse.tile as tile
    from concourse.masks import make_identity

    P1, P2 = _register_exp_ops()

    f32 = mybir.dt.float32
    bf16 = mybir.dt.bfloat16
    Exp = mybir.ActivationFunctionType.Exp
    mult = mybir.AluOpType.mult

    nc = bass.Bass("TRN2", target_bir_lowering=False, debug=False)

    xT_d = nc.dram_tensor("xT", [D, BT], bf16, kind="ExternalInput").ap()
    wq_d = nc.dram_tensor("wq", [128, 8, 128], bf16, kind="ExternalInput").ap()
    wk_d = nc.dram_tensor("wk", [128, 8, 128], bf16, kind="ExternalInput").ap()
    wv_d = nc.dram_tensor("wv", [128, 8, 128], bf16, kind="ExternalInput").ap()
    wo_d = nc.dram_tensor("wo", [DC, D], bf16, kind="ExternalInput").ap()
    mk_d = nc.dram_tensor("masks", [128, 128], bf16, kind="ExternalInput").ap()
    out_d = nc.dram_tensor("out", [BT, D], bf16, kind="ExternalOutput").ap()

    def _split_waits():
        """walrus codegen allows a single sync-wait on several instruction
        encodings (self-loading matmul LW structs, DMA triggers, extended
        DVE insts); hoist extra waits onto same-engine NoOps."""
        for bb in nc.m.functions[0].blocks:
            out = []
            for ins in bb.instructions:
                si = getattr(ins, "sync_info", None)
                if si and len(si.on_wait) > 1:
                    for i, w in enumerate(list(si.on_wait[:-1])):
                        nop = mybir.InstNoOp(
                            name=f"{ins.name}-ws{i}",
                            engine=ins.engine,
                            sync_info=mybir.SyncInfo(on_wait=[w], on_update=[]),
                        )
                        nc.register_instruction(nop)
                        out.append(nop)
                    si.on_wait = [si.on_wait[-1]]
                out.append(ins)
            bb.instructions = out

    with tile.TileContext(nc) as tc:
        with (
            tc.tile_pool(name="const", bufs=1) as constp,
            tc.tile_pool(name="wpool", bufs=1) as wpool,
            tc.tile_pool(name="resid", bufs=1) as resid,
            tc.tile_pool(name="xs", bufs=2) as xsp,
            tc.tile_pool(name="et", bufs=6) as etp,
            tc.tile_pool(name="pf", bufs=3) as pfp,
            tc.tile_pool(name="oT", bufs=2) as oTp,
            tc.tile_pool(name="dn", bufs=2) as dnp,
            tc.tile_pool(name="rb", bufs=2) as rbp,
            tc.tile_pool(name="ostg", bufs=6) as ostgp,
        ):
            ident_f = constp.tile([128, 128], f32)
            make_identity(nc, ident_f[:])
            ident_b = constp.tile([128, 128], bf16)
            nc.vector.tensor_copy(out=ident_b[:], in_=ident_f[:])
            mask_b = constp.tile([128, 128], bf16)
            # denom-broadcast selector rows (lane-matched to the AV psum
            # layout): row 64 (head0's denom partition) spreads onto
            # parts 0:64, row 0 (head1's) onto parts 64:128.
            sel_t = constp.tile([65, 128], bf16)
            nc.vector.memset(sel_t[:], 0.0)
            nc.vector.memset(sel_t[64:65, 0:64], 1.0)
            nc.vector.memset(sel_t[0:1, 64:128], 1.0)
            # C3 scalar for EXP2P_ANT (4th poly coefficient via Src1)
            c3t = constp.tile([128, 1], f32)
            nc.vector.memset(c3t[:], EXP_C[3])
            # preload the ACT exp table while DMAs run
            dum_b = constp.tile([128, 4], bf16)
            nc.scalar.activation(dum_b[:], ident_f[:, 0:4], Exp, scale=1.0)

            wq_s = wpool.tile([128, 8, 128], bf16, tag="wq")
            wk_s = wpool.tile([128, 8, 128], bf16, tag="wk")
            wv_s = wpool.tile([128, 8, 128], bf16, tag="wv")
            # qkv weights on the sync hwdge queue (critical path to first
            # matmul); masks/wo on the scalar queue.
            nc.sync.dma_start(wq_s[:], wq_d[:])
            nc.scalar.dma_start(wk_s[:], wk_d[:])
            nc.sync.dma_start(wv_s[:], wv_d[:])
            wo2_s = wpool.tile([DC, D], bf16, tag="wo")

            # qkv_T[:, 0]=Q^T, [:,1]=K^T, [:,2]=V^T, each [128, BT]
            qkv_T = resid.tile([128, 3, BT], bf16, tag="qkvT")
            qT = qkv_T[:, 0]
            kT = qkv_T[:, 1]
            vT = qkv_T[:, 2]
            # Per s-tile layout (width 193):
            #   cols   0:64  V_h0            -> head0 AV lhsT = cols 0:65
            #   col      64  ones (h0 denom)    (O_h0 at psum parts 0:64,
            #   col      65  ones (h1 denom)     denom at part 64)
            #   cols 66:129  zeros           -> head1 AV lhsT = cols 65:193
            #   cols 129:193 V_h1               (denom at psum part 0, O_h1
            #                                    at psum parts 64:128)
            v_ext = resid.tile([128, 2 * NST, 193], bf16, tag="vext")
            nc.vector.memset(v_ext[:, :, 64:129], 0.0)
            nc.vector.memset(v_ext[:, :, 64:66], 1.0)

            # ---- stage A: QKV^T projections for BATCH 0 ONLY (2 x-chunks
            # of 1024), V 128x128 PE-transposes fused in.  Batch 1's QKV
            # is deferred into batch 0's attention window as PE filler
            # units (the attention there is exp/drain-bound, PE has slack)
            xs_of = {}
            with (
                tc.tile_pool(name="qkvps", bufs=2, space="PSUM") as qkvps,
                tc.tile_pool(name="trps", bufs=2, space="PSUM") as trps,
            ):
                # warm the PE clock gate during the first x-chunk DMA:
                # HAM unthrottles after ~3.4us of sustained matmul activity
                ps_w = qkvps.tile([128, 3, TCH], f32, tag="pqkv", name="ps_warm")
                for _ in range(72):
                    nc.tensor.matmul(
                        ps_w[:, 0, 0:128], ident_b[:], ident_b[:],
                        start=True, stop=True,
                    )
                for tcix in range(BT // XCH):
                    # split each 2MB x-chunk across both hwdge queues
                    xs = xsp.tile([128, 8, XCH], bf16, tag="xs")
                    src = xT_d[:, tcix * XCH : (tcix + 1) * XCH].rearrange(
                        "(a p) t -> p a t", p=128
                    )
                    if tcix == 0:
                        nc.sync.dma_start(xs[:, 0:4, 0:TCH], src[:, 0:4, 0:TCH])
                        nc.scalar.dma_start(xs[:, 4:8, 0:TCH], src[:, 4:8, 0:TCH])
                        nc.sync.dma_start(xs[:, 0:4, TCH:XCH], src[:, 0:4, TCH:XCH])
                        nc.scalar.dma_start(xs[:, 4:8, TCH:XCH], src[:, 4:8, TCH:XCH])
                        nc.scalar.dma_start(mask_b[:], mk_d[:])
                    else:
                        nc.sync.dma_start(xs[:, 0:4], src[:, 0:4])
                        nc.scalar.dma_start(xs[:, 4:8], src[:, 4:8])
                    if tcix == 1:
                        nc.scalar.dma_start(wo2_s[:], wo_d[:])
                    for sub in range(XCH // TCH):
                        coff = tcix * XCH + sub * TCH
                        xsl = slice(sub * TCH, (sub + 1) * TCH)
                        ps_qkv = qkvps.tile([128, 3, TCH], f32, tag="pqkv")
                        for kk in range(8):
                            fl = dict(start=(kk == 0), stop=(kk == 7))
                            nc.tensor.matmul(
                                ps_qkv[:, 0], wq_s[:, kk], xs[:, kk, xsl], **fl
                            )
                            nc.tensor.matmul(
                                ps_qkv[:, 1], wk_s[:, kk], xs[:, kk, xsl], **fl
                            )
                            nc.tensor.matmul(
                                ps_qkv[:, 2], wv_s[:, kk], xs[:, kk, xsl], **fl
                            )
                        # single 3-bank cast; ACT takes the v_ext copies
                        nc.vector.tensor_copy(
                            out=qkv_T[:, :, coff : coff + TCH], in_=ps_qkv[:]
                        )
                        for stsub in range(TCH // 128):
                            st = coff // 128 + stsub
                            ps_t = trps.tile([128, 128], bf16)
                            nc.tensor.transpose(
                                ps_t[:],
                                vT[:, st * 128 : (st + 1) * 128],
                                ident_b[:],
                            )
                            nc.scalar.copy(
                                out=v_ext[:, st, 0:64], in_=ps_t[:, 0:64]
                            )
                            nc.scalar.copy(
                                out=v_ext[:, st, 129:193], in_=ps_t[:, 64:128]
                            )

            # ---- stage B: attention + fused normalize, with a generic PE
            # filler-unit queue.  The exp wall (ACT ~1.05us per full
            # s-tile) sets the si cadence; the PE has spare cycles per si
            # which filler units use, keeping the PE dense (HAM warm).
            # Units: batch 1's QKV projection (during batch 0's attention)
            # and the output projections (available once a t-chunk's
            # normalize is emitted; leftovers run in a short tail). ----
            fill_q = []       # pending unit closures
            fill_avail = 0    # prefix of fill_q that may be emitted
            fill_emit = 0     # next unit index to emit
            mc_flip = [0, 2]
            tail_mode = [False]

            def _push_oproj(b, tt, opp):
                boff = b * T
                tts = slice(tt * 128, (tt + 1) * 128)
                oT2 = oT2_of[b]
                stg = ostgp.tile([128, 2, 512], bf16, tag="stg", name=f"stg_{b}_{tt}")

                def emit_mc(mc):
                    if tail_mode[0]:
                        ps_full = sps.tile([128, HPC, TCH], f32, tag="s",
                                           name=f"tailps_{b}_{tt}_{mc}")
                        ps_out = ps_full[:, 0]
                    else:
                        ps_out = opp.tile([128, TCH], f32, tag="u")
                    nc.tensor.matmul(
                        ps_out[:],
                        oT2[:, tts],
                        wo2_s[:, mc * 512 : (mc + 1) * 512],
                        start=True,
                        stop=True,
                    )
                    # drain-engine policy: ACT every mc_flip[1]-th
                    # cast, DVE otherwise (ACT is exp-bound in batch 1)
                    if mc_flip[0] % mc_flip[1] == mc_flip[1] - 1:
                        nc.scalar.copy(out=stg[:, mc], in_=ps_out[:])
                    else:
                        nc.vector.tensor_copy(out=stg[:, mc], in_=ps_out[:])
                    mc_flip[0] += 1
                    if mc == 1:
                        dq = nc.sync if tt % 2 == 0 else nc.scalar
                        dq.dma_start(
                            out=out_d[
                                boff + tt * 128 : boff + (tt + 1) * 128, :
                            ].rearrange("t (c m) -> t c m", c=2),
                            in_=stg[:],
                        )

                fill_q.append(lambda: emit_mc(0))
                fill_q.append(lambda: emit_mc(1))

            def _push_b1_qkv(opp):
                """Batch 1 QKV as filler units: per 512-sub, one unit per
                projection (8 accumulating MMs + drain cast), then two
                V-transpose units (2 transposes + v_ext copies each)."""
                w_of = [wq_s, wk_s, wv_s]

                def mk_proj(tcix, sub, p):
                    def emit():
                        coff = tcix * XCH + sub * TCH
                        xsl = slice(sub * TCH, (sub + 1) * TCH)
                        ps_u = opp.tile([128, TCH], f32, tag="u")
                        for kk in range(8):
                            nc.tensor.matmul(
                                ps_u[:], w_of[p][:, kk],
                                xs_of[tcix][:, kk, xsl],
                                start=(kk == 0), stop=(kk == 7),
                            )
                        if p % 2 == 0:
                            nc.vector.tensor_copy(
                                out=qkv_T[:, p, coff : coff + TCH], in_=ps_u[:]
                            )
                        else:
                            nc.scalar.copy(
                                out=qkv_T[:, p, coff : coff + TCH], in_=ps_u[:]
                            )
                    return emit

                def mk_trans(tcix, sub, half):
                    def emit():
                        coff = tcix * XCH + sub * TCH
                        for stsub in range(2 * half, 2 * half + 2):
                            st = coff // 128 + stsub
                            ps_t = opp.tile([128, 128], bf16, tag="u")
                            nc.tensor.transpose(
                                ps_t[:],
                                vT[:, st * 128 : (st + 1) * 128],
                                ident_b[:],
                            )
                            nc.scalar.copy(
                                out=v_ext[:, st, 0:64], in_=ps_t[:, 0:64]
                            )
                            nc.scalar.copy(
                                out=v_ext[:, st, 129:193], in_=ps_t[:, 64:128]
                            )
                    return emit

                for tcix in (2, 3):
                    for sub in range(XCH // TCH):
                        for p in range(3):
                            fill_q.append(mk_proj(tcix, sub, p))
                        for half in range(2):
                            fill_q.append(mk_trans(tcix, sub, half))

            oT2_of = {}
            with (
                tc.tile_pool(name="sps", bufs=2, space="PSUM") as sps,
                tc.tile_pool(name="ops", bufs=1, space="PSUM") as ops,
                tc.tile_pool(name="bcp", bufs=1, space="PSUM") as bcp,
                tc.tile_pool(name="opp", bufs=1, space="PSUM") as opp,
            ):
              for b in range(B):
                mc_flip[1] = 2 if b == 0 else 3
                boff = b * T
                bst = b * NST
                # both heads' normalized O^T stacked: rows 0:64 = head 0,
                # 64:128 = head 1 -> K=128 output projection
                oT2_of[b] = oTp.tile([128, T], bf16, tag="oT2", name=f"oT2_{b}")
                oT2 = oT2_of[b]
                # denoms: head0's at partition 64, head1's at partition 0.
                # Rows 1:64 feed the K=65 broadcast matmul as zeros.
                dn2 = dnp.tile([65, T], bf16)
                nc.vector.memset(dn2[0:64, :], 0.0)
                if True:
                    pending_norm = None
                    for tj in range(NTCH):
                        tsl0 = boff + tj * TCH
                        csl = slice(tj * TCH, (tj + 1) * TCH)
                        nsi = 4 * tj + 4
                        ps_o2 = ops.tile([128, HPC, TCH], f32, tag="o")
                        avq = []   # AV emission lags scores by 2 s-tiles
                        for si in range(nsi):
                            r = si - 4 * tj
                            ssl = slice(boff + si * 128, boff + (si + 1) * 128)
                            c0 = 128 * r if r >= 1 else 0
                            nsl = slice(c0, TCH)
                            tnsl = slice(tsl0 + c0, tsl0 + TCH)
                            # scores: both heads, disjoint row groups
                            ps_s = sps.tile([128, HPC, TCH], f32, tag="s")
                            for h in range(HPC):
                                hs = slice(h * 64, (h + 1) * 64)
                                nc.tensor.matmul(
                                    ps_s[:, h, nsl],
                                    kT[hs, ssl],
                                    qT[hs, tnsl],
                                    start=True,
                                    stop=(r < 0),
                                )
                            if r >= 0:
                                # additive causal triangle (0 / -448) on the
                                # ragged diagonal block
                                for h in range(HPC):
                                    nc.tensor.matmul(
                                        ps_s[:, h, c0 : c0 + 128],
                                        ident_b[:],
                                        mask_b[:],
                                        start=False,
                                        stop=True,
                                    )
                            if si == 1 and pending_norm is not None:
                                pending_norm()
                                pending_norm = None
                                fill_avail = len(fill_q)
                            # exp: DVE custom 2-pass for diag r>=1, ACT else
                            et = etp.tile([128, HPC, TCH], bf16)
                            if _use_dve_exp(b, tj, r):
                                pf = pfp.tile([128, HPC, TCH], f32)
                                nc.vector._custom_dve(
                                    P1, out=pf[:, :, nsl], in0=ps_s[:, :, nsl],
                                    in1=c3t[:], s0=EXP_C[0], s1=EXP_C[1],
                                    imm2=EXP_C[2],
                                )
                                nc.vector._custom_dve(
                                    P2, out=et[:, :, nsl], in0=pf[:, :, nsl]
                                )
                            else:
                                nc.scalar.activation(
                                    et[:, :, nsl], ps_s[:, :, nsl], Exp,
                                    scale=0.125,
                                )
                            # filler units, BEFORE the AV so a stalled AV
                            # doesn't head-of-line block them; an extra
                            # unit at si==1 covers the AV-psum WAR stall
                            # at the tj boundary.
                            for _ in range(3 if si == 1 else (2 if (b == 1 and tj >= 2) else 1)):
                                if fill_emit >= fill_avail:
                                    break
                                fill_q[fill_emit]()
                                fill_emit += 1
                            avq.append((si, et, nsl))
                            if len(avq) > 2:
                                psi, pet, pnsl = avq.pop(0)
                                avl = [
                                    v_ext[:, bst + psi, 0:65],
                                    v_ext[:, bst + psi, 65:193],
                                ]
                                for h in range(HPC):
                                    nc.tensor.matmul(
                                        ps_o2[0 : avl[h].shape[-1], h, pnsl],
                                        avl[h],
                                        pet[:, h, pnsl],
                                        start=(psi == 0),
                                        stop=False,
                                    )
                        # drain the AV lag queue; last closes the group
                        while avq:
                            psi, pet, pnsl = avq.pop(0)
                            avl = [
                                v_ext[:, bst + psi, 0:65],
                                v_ext[:, bst + psi, 65:193],
                            ]
                            for h in range(HPC):
                                nc.tensor.matmul(
                                    ps_o2[0 : avl[h].shape[-1], h, pnsl],
                                    avl[h],
                                    pet[:, h, pnsl],
                                    start=(psi == 0),
                                    stop=(not avq),
                                )
                        # denominator slivers -> dn2 (DVE)
                        nc.vector.tensor_copy(
                            out=dn2[64:65, csl], in_=ps_o2[64:65, 0]
                        )
                        nc.vector.tensor_copy(
                            out=dn2[0:1, csl], in_=ps_o2[0:1, 1]
                        )

                        def _norm(b=b, tj=tj, csl=csl, ps_o2=ps_o2,
                                  dn2=dn2, oT2=oT2, opp=opp):
                            # PE-broadcast both denom rows in ONE K=65
                            # matmul, reciprocal, then fused drain:
                            # oT2 = ps_o2 * rb (cast to bf16)
                            ps_b = bcp.tile([128, TCH], f32)
                            nc.tensor.matmul(
                                ps_b[:], sel_t[:, :], dn2[:, csl],
                                start=True, stop=True,
                            )
                            rb = rbp.tile([128, TCH], f32)
                            nc.vector.reciprocal_approx_fast(rb[:], ps_b[:])
                            nc.vector.tensor_tensor(
                                out=oT2[0:64, csl], in0=ps_o2[0:64, 0],
                                in1=rb[0:64], op=mult,
                            )
                            nc.vector.tensor_tensor(
                                out=oT2[64:128, csl], in0=ps_o2[64:128, 1],
                                in1=rb[64:128], op=mult,
                            )
                            # tj's oproj tiles become available for filler
                            # AFTER its normalize is emitted
                            for tt in range(4 * tj, 4 * tj + 4):
                                _push_oproj(b, tt, opp)

                        pending_norm = _norm
                        # allow filler to pick up units whose norm was
                        # emitted at this tj's si==1 flush
                        fill_avail = len(fill_q)
                    # tj=3's normalize must flush inside the psum scope
                    pending_norm()
                    pending_norm = None
                    fill_avail = len(fill_q)
                    if b == B - 1:
                        # drain all remaining filler units in the tail,
                        # rotating psum across opp + the idle score slots
                        tail_mode[0] = True
                        mc_flip[1] = 2
                        k = 0
                        while fill_emit < fill_avail:
                            tail_mode[0] = (k % 3 != 0)
                            fill_q[fill_emit]()
                            fill_emit += 1
                            k += 1
    _split_waits()
    # custom-DVE ops are extended-inst InstISA subclasses whose .instr
    # bytes are populated by this pass.
    from concourse.library_overlay import lower_extended_insts

    lower_extended_insts(nc)
    return nc


def _masks_np():
    """[128,128] additive causal triangle: 0 where j>=sp, else MASKVAL."""
    sp = np.arange(128)[:, None]
    j = np.arange(128)[None, :]
    return np.where(j >= sp, 0.0, MASKVAL).astype(BF)


def kernel(x, wq, wk, wv, wo):
    from concourse.bass_utils import run_bass_kernel_spmd

    if "nc" not in _cache:
        _cache["nc"] = _build()
    nc = _cache["nc"]

    xT = np.ascontiguousarray(
        np.asarray(x, dtype=np.float32).reshape(BT, D).T
    ).astype(BF)
    wq = np.asarray(wq, dtype=np.float32)
    wk = np.asarray(wk, dtype=np.float32)
    wv = np.asarray(wv, dtype=np.float32)
    wo = np.asarray(wo, dtype=np.float32)
    masks = _masks_np()

    def _ktiles(w, cs):
        # [1024, 128] col-slice -> [p, a, n] k-tile layout, contiguous
        return np.ascontiguousarray(
            w[:, cs].reshape(8, 128, DC).transpose(1, 0, 2)
        ).astype(BF)

    in_maps = []
    for c in range(NCORES):
        cs = slice(c * DC, (c + 1) * DC)
        in_maps.append(
            {
                "xT": xT,
                "wq": _ktiles(wq, cs),
                "wk": _ktiles(wk, cs),
                "wv": _ktiles(wv, cs),
                "wo": np.ascontiguousarray(wo[cs, :]).astype(BF),
                "masks": masks,
            }
        )

    res = run_bass_kernel_spmd(
        nc, in_maps, core_ids=list(range(NCORES)), **_cache.get("run_kwargs", {})
    )
    _cache["last_res"] = res
    acc = res.results[0]["out"].astype(np.float32)
    for c in range(1, NCORES):
        acc = acc + res.results[c]["out"].astype(np.float32)
    return acc.reshape(B, T, D)
